# revision 1
# baseline (speedup 1.0000x reference)
"""CosineAttention on 8 TRN2 NeuronCores.

Sharding (head + tensor parallel, per the hint):
  core c owns head h=c for both batches:
    - computes qT,kT = [Wq_h|Wk_h]^T-stationary matmuls over full xT
    - RMS-normalizes q,k in the [d, i] layout via a PE ones-matmul
      partition-reduction + K=2 outer-product broadcast
    - simT[j,i] = kn^T qn (K=64, float32r), exp folded scale=1/8 on ACT
    - attn@v with a ones column appended to v so the softmax denominator
      falls out as matmul row 64; normalize by its reciprocal
    - per-batch AllGather of o_cT [64, n] (bf16) -> [512, n] feature-major
    - column-parallel out-proj: outT_c [64, n] = W2_c^T-stationary matmul
  host concatenates the 8 feature slices.

Matmul dtype: float32r (TF32-class single-pass PE mode) for the fp32 path;
bf16 for the post-softmax path (attn weights / v / out-proj operands).
"""

import numpy as np
import ml_dtypes

import concourse.bass as bass
import concourse.tile as tile
from concourse import bacc
import concourse.mybir as mybir
from concourse import bass_utils

f32 = mybir.dt.float32
f32r = mybir.dt.float32r
bf16 = mybir.dt.bfloat16
AF = mybir.ActivationFunctionType

N_CORES = 8
HEADS = 8
D = 64            # head dim
B = 2             # batch
SEQ = 2048        # tokens per batch
DIM = 512         # model dim = HEADS * D
NTOK = B * SEQ    # 4096
EPS = 1e-4
SCALE = D ** -0.5  # 0.125

FT = DIM // 128   # 4 f-tiles of 128
CH1 = 512         # stage-1 token chunk
NCH1 = NTOK // CH1            # 8
ICH = 1024        # phase-2 i-chunk (exp batching)
NICH = SEQ // ICH             # 2 per batch
JT = SEQ // 128   # 16 j-tiles per batch
PCH = 512         # phase-3 chunk
NPCH = SEQ // PCH             # 4 per batch

_BUILD_CACHE = {}


def build(collective=True, num_devices=N_CORES, reps=1):
    key = (collective, num_devices, reps)
    if key in _BUILD_CACHE:
        return _BUILD_CACHE[key]
    nc = bacc.Bacc("TRN2", target_bir_lowering=False, debug=False,
                   num_devices=num_devices)
    xT = nc.dram_tensor("xT", [DIM, NTOK], f32, kind="ExternalInput").ap()
    wqk = nc.dram_tensor("wqk", [DIM, 128], f32, kind="ExternalInput").ap()
    wv = nc.dram_tensor("wv", [DIM, D], f32, kind="ExternalInput").ap()
    w2 = nc.dram_tensor("w2", [DIM, D], bf16, kind="ExternalInput").ap()
    ones2 = nc.dram_tensor("ones2", [128, 2], f32, kind="ExternalInput").ap()
    e2 = nc.dram_tensor("e2", [2, 128], f32, kind="ExternalInput").ap()
    ones1 = nc.dram_tensor("ones1", [1, D], f32, kind="ExternalInput").ap()
    outT = nc.dram_tensor("outT", [D, NTOK], f32, kind="ExternalOutput").ap()

    with tile.TileContext(nc) as tc:
        with (
            tc.tile_pool(name="persist", bufs=1) as pp,
            tc.tile_pool(name="sb", bufs=2) as sb,
            tc.tile_pool(name="ps", bufs=1, space="PSUM") as ps,
            tc.tile_pool(name="dram", bufs=1, space="DRAM") as dram,
            nc.allow_low_precision(reason="f32r matmuls; bf16 attn/out path"),
        ):
            # ---- persistent weights / constants ----
            wqk_sb = pp.tile([128, FT, 128], f32r)
            wv_sb = pp.tile([128, FT, D], f32r)
            w2_sb = pp.tile([128, FT, D], bf16)
            for t in range(FT):
                nc.sync.dma_start(wqk_sb[:, t, :],
                                  wqk[t * 128:(t + 1) * 128, :].bitcast(f32r))
                nc.sync.dma_start(wv_sb[:, t, :],
                                  wv[t * 128:(t + 1) * 128, :].bitcast(f32r))
                nc.sync.dma_start(w2_sb[:, t, :], w2[t * 128:(t + 1) * 128, :])
            o2_sb = pp.tile([128, 2], f32r)
            nc.sync.dma_start(o2_sb[:], ones2[:].bitcast(f32r))
            e2_sb = pp.tile([2, 128], f32r)
            nc.sync.dma_start(e2_sb[:], e2[:].bitcast(f32r))
            o1_sb = pp.tile([1, D], f32r)
            nc.sync.dma_start(o1_sb[:], ones1[:].bitcast(f32r))

            # ---- persistent activations ----
            qn_sb = pp.tile([D, NTOK], f32r)     # normalized qT
            kn_sb = pp.tile([D, NTOK], f32r)     # normalized kT
            qk_all = pp.tile([128, NTOK], f32r)  # raw [q;k]T
            vo_sb = pp.tile([128, NTOK // 128, D + 1], bf16)  # v | ones

            # ---- stage 1: qkv projections + rms normalization ----
            for rep in range(reps):
              for ci in range(NCH1):
                  cols = slice(ci * CH1, (ci + 1) * CH1)
                  xt_sb = sb.tile([128, FT, CH1], f32r, tag="xt")
                  for t in range(FT):
                      nc.sync.dma_start(
                          xt_sb[:, t, :],
                          xT[t * 128:(t + 1) * 128, cols].bitcast(f32r))

                  # qkT chunk: [q;k] x-stream, W stationary
                  qk_ps = ps.tile([128, CH1], f32, tag="small", bufs=4)
                  for t in range(FT):
                      nc.tensor.matmul(qk_ps[:], wqk_sb[:, t, :], xt_sb[:, t, :],
                                       start=(t == 0), stop=(t == FT - 1))
                  # v chunk: [j, d], xT stationary
                  for js in range(CH1 // 128):
                      v_ps = ps.tile([128, D], f32, tag="small", bufs=4)
                      for t in range(FT):
                          nc.tensor.matmul(
                              v_ps[:],
                              xt_sb[:, t, js * 128:(js + 1) * 128],
                              wv_sb[:, t, :],
                              start=(t == 0), stop=(t == FT - 1))
                      jt = ci * (CH1 // 128) + js
                      nc.vector.tensor_copy(vo_sb[:, jt, 0:D], v_ps[:])
                      nc.gpsimd.memset(vo_sb[:, jt, D:D + 1], 1.0)

                  # raw qk to sbuf (ACT; DVE is busier)
                  nc.scalar.activation(qk_all[:, cols], qk_ps[:], AF.Copy)
                  # sq = qk^2 (DVE, from the sbuf copy)
                  sq_sb = sb.tile([128, CH1], f32r, tag="sq")
                  nc.vector.tensor_mul(sq_sb[:], qk_all[:, cols],
                                       qk_all[:, cols])
                  # st[2, CH1] = column sums of q-sq and k-sq
                  st_ps = ps.tile([2, CH1], f32, tag="small", bufs=4)
                  nc.tensor.matmul(st_ps[:], o2_sb[:], sq_sb[:],
                                   start=True, stop=True)
                  # r = 1/(sqrt(st/64) + eps)
                  rt_sb = sb.tile([2, CH1], f32, tag="rt")
                  nc.scalar.activation(rt_sb[:], st_ps[:], AF.Sqrt,
                                       scale=1.0 / D)
                  re_sb = sb.tile([2, CH1], f32, tag="re")
                  nc.vector.tensor_scalar_add(re_sb[:], rt_sb[:], EPS)
                  rc_sb = sb.tile([2, CH1], f32r, tag="rc")
                  nc.vector.reciprocal(rc_sb[:], re_sb[:])
                  # R[128, CH1] = outer(e2, r): row broadcast of scales
                  r_ps = ps.tile([128, CH1], f32, tag="small", bufs=4)
                  nc.tensor.matmul(r_ps[:], e2_sb[:], rc_sb[:],
                                   start=True, stop=True)
                  rb_sb = sb.tile([128, CH1], f32r, tag="rb")
                  nc.scalar.activation(rb_sb[:], r_ps[:], AF.Copy)
                  # apply
                  nc.vector.tensor_mul(qn_sb[:, cols], qk_all[0:D, cols],
                                       rb_sb[0:D, :])
                  nc.vector.tensor_mul(kn_sb[:, cols], qk_all[D:128, cols],
                                       rb_sb[D:128, :])

              # ---- per-batch: attention + allgather + out-proj ----
              cc_outs = []
              for b in range(B):
                  cc_in = dram.tile([D, SEQ], bf16, name=f"cc_in{b}")
                  cc_out = dram.tile([DIM, SEQ], bf16, addr_space="Shared",
                                     name=f"cc_out{b}")
                  cc_outs.append(cc_out)
                  for ic in range(NICH):
                      i0 = b * SEQ + ic * ICH
                      expT = sb.tile([128, JT, ICH], bf16, tag="expT")
                      for jt in range(JT):
                          j0 = b * SEQ + jt * 128
                          sim_ps = ps.tile([128, ICH], f32, tag="big", bufs=2)
                          for h in range(ICH // 512):
                              nc.tensor.matmul(
                                  sim_ps[:, h * 512:(h + 1) * 512],
                                  kn_sb[:, j0:j0 + 128],
                                  qn_sb[:, i0 + h * 512:i0 + (h + 1) * 512],
                                  start=True, stop=True)
                          nc.scalar.activation(expT[:, jt, :], sim_ps[:],
                                               AF.Exp, scale=SCALE)
                      for h in range(ICH // 512):
                          av_ps = ps.tile([D + 1, 512], f32, tag="small", bufs=4)
                          for jt in range(JT):
                              nc.tensor.matmul(
                                  av_ps[:],
                                  vo_sb[:, b * JT + jt, :],
                                  expT[:, jt, h * 512:(h + 1) * 512],
                                  start=(jt == 0), stop=(jt == JT - 1))
                          # normalize by sumexp (row D) and emit bf16
                          rse_sb = sb.tile([1, 512], f32r, tag="rse")
                          nc.vector.reciprocal(rse_sb[:],
                                               av_ps[D:D + 1, :].bitcast(f32r))
                          r2_ps = ps.tile([D, 512], f32, tag="small", bufs=4)
                          nc.tensor.matmul(r2_ps[:], o1_sb[:], rse_sb[:],
                                           start=True, stop=True)
                          r2_sb = sb.tile([D, 512], f32, tag="r2")
                          nc.scalar.activation(r2_sb[:], r2_ps[:], AF.Copy)
                          oc_sb = sb.tile([D, 512], bf16, tag="oc")
                          nc.vector.tensor_mul(oc_sb[:], av_ps[0:D, :], r2_sb[:])
                          nc.sync.dma_start(
                              cc_in[:, ic * ICH + h * 512:
                                    ic * ICH + (h + 1) * 512], oc_sb[:])
                  if collective:
                      nc.gpsimd.collective_compute(
                          "AllGather", mybir.AluOpType.bypass,
                          replica_groups=[list(range(num_devices))],
                          ins=[cc_in[:]], outs=[cc_out[:]])
                  else:
                      # timing-only stand-in: keep the DRAM write traffic
                      nc.sync.dma_start(cc_out[0:D, :], cc_in[:])

              for b in range(B):
                  cc_out = cc_outs[b]
                  for pc in range(NPCH):
                      cols = slice(pc * PCH, (pc + 1) * PCH)
                      ag_sb = sb.tile([128, FT, PCH], bf16, tag="ag")
                      for t in range(FT):
                          nc.sync.dma_start(ag_sb[:, t, :],
                                            cc_out[t * 128:(t + 1) * 128, cols])
                      fp_ps = ps.tile([D, PCH], f32, tag="small", bufs=4)
                      for t in range(FT):
                          nc.tensor.matmul(fp_ps[:], w2_sb[:, t, :],
                                           ag_sb[:, t, :],
                                           start=(t == 0), stop=(t == FT - 1))
                      fo_sb = sb.tile([D, PCH], f32, tag="fo")
                      nc.vector.tensor_copy(fo_sb[:], fp_ps[:])
                      nc.sync.dma_start(
                          outT[:, b * SEQ + pc * PCH:b * SEQ + (pc + 1) * PCH],
                          fo_sb[:])
    nc.compile()
    _BUILD_CACHE[key] = nc
    return nc


def make_in_maps(x, Wq, Wkv, Wout):
    xT = np.ascontiguousarray(x.reshape(NTOK, DIM).T).astype(np.float32)
    ones2 = np.zeros((128, 2), np.float32)
    ones2[0:D, 0] = 1.0
    ones2[D:128, 1] = 1.0
    e2 = np.ascontiguousarray(ones2.T)
    ones1 = np.ones((1, D), np.float32)
    in_maps = []
    for c in range(N_CORES):
        rows = slice(c * D, (c + 1) * D)
        wqk = np.ascontiguousarray(
            np.concatenate([Wq[rows, :].T, Wkv[rows, :].T], axis=1))
        wv = np.ascontiguousarray(Wkv[DIM + c * D:DIM + (c + 1) * D, :].T)
        w2 = np.ascontiguousarray(Wout[rows, :].T).astype(ml_dtypes.bfloat16)
        in_maps.append({
            "xT": xT, "wqk": wqk.astype(np.float32),
            "wv": wv.astype(np.float32), "w2": w2,
            "ones2": ones2, "e2": e2, "ones1": ones1,
        })
    return in_maps


def kernel(x, Wq, Wkv, Wout, _trace=False):
    nc = build()
    in_maps = make_in_maps(np.asarray(x), np.asarray(Wq), np.asarray(Wkv),
                           np.asarray(Wout))
    res = bass_utils.run_bass_kernel_spmd(
        nc, in_maps, core_ids=list(range(N_CORES)), trace=_trace)
    out = np.empty((NTOK, DIM), np.float32)
    for c in range(N_CORES):
        out[:, c * D:(c + 1) * D] = res.results[c]["outT"].T
    full = out.reshape(B, SEQ, DIM)
    if _trace:
        return full, res
    return full



# revision 8
# speedup vs baseline: 1.0149x; 1.0149x over previous
"""CosineAttention on 8 TRN2 NeuronCores.

Sharding (head + tensor parallel, per the hint):
  core c owns head h=c for both batches:
    - computes qT,kT = [Wq_h|Wk_h]^T-stationary matmuls over full xT
    - RMS-normalizes q,k in the [d, i] layout: PE ones-matmul partition
      reduction -> K=2 outer-product broadcast -> single ACT Rsqrt
    - simT[j,i] = kn^T qn (K=64, float32r), exp folded scale=1/8 on ACT
    - attn@v with a ones column appended to v so the softmax denominator
      falls out as matmul row 64; normalize via DVE reciprocal_approx_fast
      + PE broadcast + GpSimd copy + DVE multiply
    - per-(batch, 1024-slab) AllGather of o_cT [64, 1024] (bf16)
      -> [512, 1024] feature-major; out-proj fused right after so it
      overlaps the next slab's attention
    - column-parallel out-proj: outT_c [64, n] = W2_c^T-stationary matmul,
      DMA'd to DRAM straight from PSUM
  host concatenates the 8 feature slices.

Matmul dtype: float32r (TF32-class single-pass PE mode) for the fp32 path;
bf16 for the post-softmax path (attn weights / v / out-proj operands).
"""

import numpy as np
import ml_dtypes

import concourse.bass as bass
import concourse.tile as tile
from concourse import bacc
import concourse.mybir as mybir
from concourse import bass_utils

f32 = mybir.dt.float32
f32r = mybir.dt.float32r
bf16 = mybir.dt.bfloat16
AF = mybir.ActivationFunctionType

N_CORES = 8
HEADS = 8
D = 64            # head dim
B = 2             # batch
SEQ = 2048        # tokens per batch
DIM = 512         # model dim = HEADS * D
NTOK = B * SEQ    # 4096
SCALE = D ** -0.5  # 0.125

FT = DIM // 128   # 4 f-tiles of 128
CH1 = 512         # stage-1 token chunk
NCH1 = NTOK // CH1            # 8
ICH = 1024        # phase-2 i-slab (exp batching + collective granularity)
NICH = SEQ // ICH             # 2 per batch
JT = SEQ // 128   # 16 j-tiles per batch
PCH = 512         # phase-3 chunk
NPCH = SEQ // PCH             # 4 per batch

_BUILD_CACHE = {}


def build(collective=True, num_devices=N_CORES):
    key = (collective, num_devices)
    if key in _BUILD_CACHE:
        return _BUILD_CACHE[key]
    nc = bacc.Bacc("TRN2", target_bir_lowering=False, debug=False,
                   num_devices=num_devices)
    xT = nc.dram_tensor("xT", [DIM, NTOK], f32, kind="ExternalInput").ap()
    wqk = nc.dram_tensor("wqk", [DIM, 128], f32, kind="ExternalInput").ap()
    wv = nc.dram_tensor("wv", [DIM, D], f32, kind="ExternalInput").ap()
    w2 = nc.dram_tensor("w2", [DIM, D], bf16, kind="ExternalInput").ap()
    ones2 = nc.dram_tensor("ones2", [128, 2], f32, kind="ExternalInput").ap()
    e2 = nc.dram_tensor("e2", [2, 128], f32, kind="ExternalInput").ap()
    ones1 = nc.dram_tensor("ones1", [1, D], f32, kind="ExternalInput").ap()
    outT = nc.dram_tensor("outT", [D, NTOK], f32, kind="ExternalOutput").ap()

    with tile.TileContext(nc) as tc:
        with (
            tc.tile_pool(name="persist", bufs=1) as pp,
            tc.tile_pool(name="sb", bufs=2) as sb,
            tc.tile_pool(name="ps", bufs=1, space="PSUM") as ps,
            tc.tile_pool(name="dram", bufs=1, space="DRAM") as dram,
            nc.allow_low_precision(reason="f32r matmuls; bf16 attn/out path"),
        ):
            # ---- persistent weights / constants ----
            wqk_sb = pp.tile([128, FT, 128], f32r)
            wv_sb = pp.tile([128, FT, D], f32r)
            w2_sb = pp.tile([128, FT, D], bf16)
            for t in range(FT):
                nc.sync.dma_start(wqk_sb[:, t, :],
                                  wqk[t * 128:(t + 1) * 128, :].bitcast(f32r))
                nc.sync.dma_start(wv_sb[:, t, :],
                                  wv[t * 128:(t + 1) * 128, :].bitcast(f32r))
                nc.sync.dma_start(w2_sb[:, t, :], w2[t * 128:(t + 1) * 128, :])
            o2_sb = pp.tile([128, 2], f32r)
            nc.sync.dma_start(o2_sb[:], ones2[:].bitcast(f32r))
            e2_sb = pp.tile([2, 128], f32r)
            nc.sync.dma_start(e2_sb[:], e2[:].bitcast(f32r))
            o1_sb = pp.tile([1, D], f32r)
            nc.sync.dma_start(o1_sb[:], ones1[:].bitcast(f32r))

            # ---- persistent activations ----
            qn_sb = pp.tile([D, NTOK], f32r)     # normalized qT
            kn_sb = pp.tile([D, NTOK], f32r)     # normalized kT
            qk_all = pp.tile([128, NTOK], f32r)  # raw [q;k]T
            vo_sb = pp.tile([128, NTOK // 128, D + 1], bf16)  # v | ones

            # ---- stage 1: qkv projections + rms normalization ----
            for ci in range(NCH1):
                cols = slice(ci * CH1, (ci + 1) * CH1)
                xt_sb = sb.tile([128, FT, CH1], f32r, tag="xt")
                for t in range(FT):
                    nc.sync.dma_start(
                        xt_sb[:, t, :],
                        xT[t * 128:(t + 1) * 128, cols].bitcast(f32r))

                # qkT chunk: [q;k] x-stream, W stationary
                qk_ps = ps.tile([128, CH1], f32, tag="small", bufs=4)
                for t in range(FT):
                    nc.tensor.matmul(qk_ps[:], wqk_sb[:, t, :], xt_sb[:, t, :],
                                     start=(t == 0), stop=(t == FT - 1))
                # v chunk: [j, d], xT stationary
                for js in range(CH1 // 128):
                    v_ps = ps.tile([128, D], f32, tag="small", bufs=4)
                    for t in range(FT):
                        nc.tensor.matmul(
                            v_ps[:],
                            xt_sb[:, t, js * 128:(js + 1) * 128],
                            wv_sb[:, t, :],
                            start=(t == 0), stop=(t == FT - 1))
                    jt = ci * (CH1 // 128) + js
                    nc.vector.tensor_copy(vo_sb[:, jt, 0:D], v_ps[:])
                    nc.gpsimd.memset(vo_sb[:, jt, D:D + 1], 1.0)

                # raw qk to sbuf (DVE; ACT keeps its table resident)
                nc.vector.tensor_copy(qk_all[:, cols], qk_ps[:])
                # sq = qk^2
                sq_sb = sb.tile([128, CH1], f32r, tag="sq")
                nc.vector.tensor_mul(sq_sb[:], qk_all[:, cols],
                                     qk_all[:, cols])
                # st[2, CH1] = column sums of q-sq and k-sq
                st_ps = ps.tile([2, CH1], f32, tag="small", bufs=4)
                nc.tensor.matmul(st_ps[:], o2_sb[:], sq_sb[:],
                                 start=True, stop=True)
                # rst = 1/st (fast approx), then broadcast, then
                # rb = sqrt(64/st) on ACT  (eps dropped: ~1e-4 rel effect)
                rstr_sb = sb.tile([2, CH1], f32r, tag="str")
                nc.vector.reciprocal(rstr_sb[:], st_ps[:].bitcast(f32r))
                r_ps = ps.tile([128, CH1], f32, tag="small", bufs=4)
                nc.tensor.matmul(r_ps[:], e2_sb[:], rstr_sb[:],
                                 start=True, stop=True)
                rb_sb = sb.tile([128, CH1], f32r, tag="rb")
                nc.scalar.activation(rb_sb[:], r_ps[:], AF.Sqrt,
                                     scale=float(D))
                # apply
                nc.vector.tensor_mul(qn_sb[:, cols], qk_all[0:D, cols],
                                     rb_sb[0:D, :])
                nc.vector.tensor_mul(kn_sb[:, cols], qk_all[D:128, cols],
                                     rb_sb[D:128, :])

            # ---- per (batch, 1024-slab): attention + allgather; then
            # ---- per batch: column-parallel out-proj ----
            cc_outs = {}
            for b in range(B):
                for ic in range(NICH):
                    cc_in = dram.tile([D, ICH], bf16, name=f"cc_in{b}_{ic}")
                    cc_out = dram.tile([DIM, ICH], bf16, addr_space="Shared",
                                       name=f"cc_out{b}_{ic}")
                    cc_outs[(b, ic)] = cc_out
                    i0 = b * SEQ + ic * ICH
                    expT = sb.tile([128, JT, ICH], bf16, tag="expT")
                    for jt in range(JT):
                        j0 = b * SEQ + jt * 128
                        sim_ps = ps.tile([128, ICH], f32, tag="big", bufs=2)
                        for h in range(ICH // 512):
                            nc.tensor.matmul(
                                sim_ps[:, h * 512:(h + 1) * 512],
                                kn_sb[:, j0:j0 + 128],
                                qn_sb[:, i0 + h * 512:i0 + (h + 1) * 512],
                                start=True, stop=True)
                        nc.scalar.activation(expT[:, jt, :], sim_ps[:],
                                             AF.Exp, scale=SCALE)
                    for h in range(ICH // 512):
                        av_ps = ps.tile([D + 1, 512], f32, tag="small", bufs=4)
                        for jt in range(JT):
                            nc.tensor.matmul(
                                av_ps[:],
                                vo_sb[:, b * JT + jt, :],
                                expT[:, jt, h * 512:(h + 1) * 512],
                                start=(jt == 0), stop=(jt == JT - 1))
                        # normalize by sumexp (row D) and emit bf16
                        rser_sb = sb.tile([1, 512], f32r, tag="rser")
                        nc.vector.reciprocal(
                            rser_sb[:], av_ps[D:D + 1, :].bitcast(f32r))
                        r2_ps = ps.tile([D, 512], f32, tag="small", bufs=4)
                        nc.tensor.matmul(r2_ps[:], o1_sb[:], rser_sb[:],
                                         start=True, stop=True)
                        r2_sb = sb.tile([D, 512], f32, tag="r2")
                        nc.vector.tensor_copy(r2_sb[:], r2_ps[:])
                        oc_sb = sb.tile([D, 512], bf16, tag="oc")
                        nc.vector.tensor_mul(oc_sb[:], av_ps[0:D, :],
                                             r2_sb[:])
                        nc.sync.dma_start(
                            cc_in[:, h * 512:(h + 1) * 512], oc_sb[:])
                    if collective:
                        nc.gpsimd.collective_compute(
                            "AllGather", mybir.AluOpType.bypass,
                            replica_groups=[list(range(num_devices))],
                            ins=[cc_in[:]], outs=[cc_out[:]])
                    else:
                        # timing-only stand-in: keep the DRAM write traffic
                        nc.sync.dma_start(cc_out[0:D, :], cc_in[:])

                for pc in range(NPCH):
                    ic, off = pc // 2, (pc % 2) * PCH
                    cc_out = cc_outs[(b, ic)]
                    ag_sb = sb.tile([128, FT, PCH], bf16, tag="ag")
                    for t in range(FT):
                        nc.sync.dma_start(
                            ag_sb[:, t, :],
                            cc_out[t * 128:(t + 1) * 128, off:off + PCH])
                    fp_ps = ps.tile([D, PCH], f32, tag="small", bufs=4)
                    for t in range(FT):
                        nc.tensor.matmul(fp_ps[:], w2_sb[:, t, :],
                                         ag_sb[:, t, :],
                                         start=(t == 0), stop=(t == FT - 1))
                    fo_sb = sb.tile([D, PCH], f32, tag="fo")
                    nc.vector.tensor_copy(fo_sb[:], fp_ps[:])
                    nc.sync.dma_start(
                        outT[:, b * SEQ + pc * PCH:b * SEQ + (pc + 1) * PCH],
                        fo_sb[:])
    nc.compile()
    _BUILD_CACHE[key] = nc
    return nc


def make_in_maps(x, Wq, Wkv, Wout):
    xT = np.ascontiguousarray(x.reshape(NTOK, DIM).T).astype(np.float32)
    ones2 = np.zeros((128, 2), np.float32)
    ones2[0:D, 0] = 1.0
    ones2[D:128, 1] = 1.0
    e2 = np.ascontiguousarray(ones2.T)
    ones1 = np.ones((1, D), np.float32)
    in_maps = []
    for c in range(N_CORES):
        rows = slice(c * D, (c + 1) * D)
        wqk = np.ascontiguousarray(
            np.concatenate([Wq[rows, :].T, Wkv[rows, :].T], axis=1))
        wv = np.ascontiguousarray(Wkv[DIM + c * D:DIM + (c + 1) * D, :].T)
        w2 = np.ascontiguousarray(Wout[rows, :].T).astype(ml_dtypes.bfloat16)
        in_maps.append({
            "xT": xT, "wqk": wqk.astype(np.float32),
            "wv": wv.astype(np.float32), "w2": w2,
            "ones2": ones2, "e2": e2, "ones1": ones1,
        })
    return in_maps


def kernel(x, Wq, Wkv, Wout, _trace=False):
    nc = build()
    in_maps = make_in_maps(np.asarray(x), np.asarray(Wq), np.asarray(Wkv),
                           np.asarray(Wout))
    res = bass_utils.run_bass_kernel_spmd(
        nc, in_maps, core_ids=list(range(N_CORES)), trace=_trace)
    out = np.empty((NTOK, DIM), np.float32)
    for c in range(N_CORES):
        out[:, c * D:(c + 1) * D] = res.results[c]["outT"].T
    full = out.reshape(B, SEQ, DIM)
    if _trace:
        return full, res
    return full


# revision 11
# speedup vs baseline: 1.0259x; 1.0108x over previous
"""CosineAttention on 8 TRN2 NeuronCores.

Sharding (head + tensor parallel, per the hint):
  core c owns head h=c for both batches:
    - computes qT,kT = [Wq_h|Wk_h]^T-stationary matmuls over full xT
    - RMS-normalizes q,k in the [d, i] layout: PE ones-matmul partition
      reduction -> ACT Sqrt -> DVE reciprocal_approx_fast (SBUF-only; the
      custom DVE op mis-reads PSUM) -> GpSimd partition_broadcast
    - simT[j,i] = kn^T qn (K=64, float32r), exp folded scale=1/8 on ACT
    - attn@v with a ones column appended to v so the softmax denominator
      falls out as matmul row 64; normalize via DVE copy +
      reciprocal_approx_fast + GpSimd partition_broadcast + DVE multiply
    - per-(batch, 512-slab) AllGather of o_cT [64, 512] (bf16)
      -> [512, 512] feature-major; out-proj fused per batch so it
      overlaps the next batch's attention
    - column-parallel out-proj: outT_c [64, n] = W2_c^T-stationary matmul
  host concatenates the 8 feature slices.

Matmul dtype: float32r (TF32-class single-pass PE mode) for the fp32 path;
bf16 for the post-softmax path (attn weights / v / out-proj operands).
"""

import numpy as np
import ml_dtypes

import concourse.bass as bass
import concourse.tile as tile
from concourse import bacc
import concourse.mybir as mybir
from concourse import bass_utils

f32 = mybir.dt.float32
f32r = mybir.dt.float32r
bf16 = mybir.dt.bfloat16
AF = mybir.ActivationFunctionType

N_CORES = 8
HEADS = 8
D = 64            # head dim
B = 2             # batch
SEQ = 2048        # tokens per batch
DIM = 512         # model dim = HEADS * D
NTOK = B * SEQ    # 4096
SCALE = D ** -0.5  # 0.125

FT = DIM // 128   # 4 f-tiles of 128
CH1 = 512         # stage-1 token chunk
NCH1 = NTOK // CH1            # 8
ICH = 1024        # phase-2 i-slab (exp batching)
NICH = SEQ // ICH             # 2 per batch
JT = SEQ // 128   # 16 j-tiles per batch
PCH = 512         # phase-3 chunk = collective slab
NPCH = SEQ // PCH             # 4 per batch

_BUILD_CACHE = {}


def build(collective=True, num_devices=N_CORES):
    key = (collective, num_devices)
    if key in _BUILD_CACHE:
        return _BUILD_CACHE[key]
    nc = bacc.Bacc("TRN2", target_bir_lowering=False, debug=False,
                   num_devices=num_devices)
    xT = nc.dram_tensor("xT", [DIM, NTOK], f32, kind="ExternalInput").ap()
    wqk = nc.dram_tensor("wqk", [DIM, 128], f32, kind="ExternalInput").ap()
    wv = nc.dram_tensor("wv", [DIM, D], f32, kind="ExternalInput").ap()
    w2 = nc.dram_tensor("w2", [DIM, D], bf16, kind="ExternalInput").ap()
    ones2 = nc.dram_tensor("ones2", [128, 2], f32, kind="ExternalInput").ap()
    e2 = nc.dram_tensor("e2", [2, 128], f32, kind="ExternalInput").ap()
    ones1 = nc.dram_tensor("ones1", [1, D], f32, kind="ExternalInput").ap()
    outT = nc.dram_tensor("outT", [D, NTOK], f32, kind="ExternalOutput").ap()

    with tile.TileContext(nc) as tc:
        with (
            tc.tile_pool(name="persist", bufs=1) as pp,
            tc.tile_pool(name="sb", bufs=2) as sb,
            tc.tile_pool(name="ps", bufs=1, space="PSUM") as ps,
            tc.tile_pool(name="dram", bufs=1, space="DRAM") as dram,
            nc.allow_low_precision(reason="f32r matmuls; bf16 attn/out path"),
        ):
            # ---- persistent weights / constants ----
            wqk_sb = pp.tile([128, FT, 128], f32r)
            wv_sb = pp.tile([128, FT, D], f32r)
            w2_sb = pp.tile([128, FT, D], bf16)
            for t in range(FT):
                nc.sync.dma_start(wqk_sb[:, t, :],
                                  wqk[t * 128:(t + 1) * 128, :].bitcast(f32r))
                nc.sync.dma_start(wv_sb[:, t, :],
                                  wv[t * 128:(t + 1) * 128, :].bitcast(f32r))
                nc.sync.dma_start(w2_sb[:, t, :], w2[t * 128:(t + 1) * 128, :])
            o2_sb = pp.tile([128, 2], f32r)
            nc.sync.dma_start(o2_sb[:], ones2[:].bitcast(f32r))
            e2_sb = pp.tile([2, 128], f32r)
            nc.sync.dma_start(e2_sb[:], e2[:].bitcast(f32r))

            # ---- persistent activations ----
            qn_sb = pp.tile([D, NTOK], f32r)     # normalized qT
            kn_sb = pp.tile([D, NTOK], f32r)     # normalized kT
            qk_all = pp.tile([128, NTOK], f32r)  # raw [q;k]T
            vo_sb = pp.tile([128, NTOK // 128, D + 1], bf16)  # v | ones

            # ---- stage 1: qkv projections + rms normalization ----
            for ci in range(NCH1):
                cols = slice(ci * CH1, (ci + 1) * CH1)
                xt_sb = sb.tile([128, FT, CH1], f32r, tag="xt")
                for t in range(FT):
                    nc.sync.dma_start(
                        xt_sb[:, t, :],
                        xT[t * 128:(t + 1) * 128, cols].bitcast(f32r))

                # qkT chunk: [q;k] x-stream, W stationary
                qk_ps = ps.tile([128, CH1], f32, tag="small", bufs=4)
                for t in range(FT):
                    nc.tensor.matmul(qk_ps[:], wqk_sb[:, t, :], xt_sb[:, t, :],
                                     start=(t == 0), stop=(t == FT - 1))
                # v chunk: [j, d], xT stationary
                for js in range(CH1 // 128):
                    v_ps = ps.tile([128, D], f32, tag="small", bufs=4)
                    for t in range(FT):
                        nc.tensor.matmul(
                            v_ps[:],
                            xt_sb[:, t, js * 128:(js + 1) * 128],
                            wv_sb[:, t, :],
                            start=(t == 0), stop=(t == FT - 1))
                    jt = ci * (CH1 // 128) + js
                    nc.vector.tensor_copy(vo_sb[:, jt, 0:D], v_ps[:])
                    nc.gpsimd.memset(vo_sb[:, jt, D:D + 1], 1.0)

                # raw qk to sbuf (DVE)
                nc.vector.tensor_copy(qk_all[:, cols], qk_ps[:])
                # sq = qk^2
                sq_sb = sb.tile([128, CH1], f32r, tag="sq")
                nc.vector.tensor_mul(sq_sb[:], qk_all[:, cols],
                                     qk_all[:, cols])
                # st[2, CH1] = column sums of q-sq and k-sq
                st_ps = ps.tile([2, CH1], f32, tag="small", bufs=4)
                nc.tensor.matmul(st_ps[:], o2_sb[:], sq_sb[:],
                                 start=True, stop=True)
                # ssq = sqrt(st/64); rst = 1/ssq  (eps dropped: ~1e-4 rel)
                ssq_sb = sb.tile([2, CH1], f32, tag="ssq")
                nc.scalar.activation(ssq_sb[:], st_ps[:], AF.Sqrt,
                                     scale=1.0 / D)
                rst_sb = sb.tile([2, CH1], f32, tag="rst")
                nc.vector.reciprocal_approx_fast(rst_sb[:], ssq_sb[:])
                rstr_sb = sb.tile([2, CH1], f32r, tag="rstr")
                nc.vector.tensor_copy(rstr_sb[:], rst_sb[:])
                # broadcast scales across head-dim partitions (PE outer
                # product); muls consume the PSUM broadcast directly
                r_ps = ps.tile([128, CH1], f32, tag="small", bufs=4)
                nc.tensor.matmul(r_ps[:], e2_sb[:], rstr_sb[:],
                                 start=True, stop=True)
                # apply
                nc.vector.tensor_mul(qn_sb[:, cols],
                                     qk_all[0:D, cols].bitcast(f32),
                                     r_ps[0:D, :])
                nc.vector.tensor_mul(kn_sb[:, cols],
                                     qk_all[D:128, cols].bitcast(f32),
                                     r_ps[D:128, :])

            # ---- per (batch, 1024-slab): attention; AllGather per 512;
            # ---- per batch: column-parallel out-proj ----
            cc_outs = {}
            for b in range(B):
                for ic in range(NICH):
                    i0 = b * SEQ + ic * ICH
                    expT = sb.tile([128, JT, ICH], bf16, tag="expT")
                    for jt in range(JT):
                        j0 = b * SEQ + jt * 128
                        sim_ps = ps.tile([128, ICH], f32, tag="big", bufs=2)
                        for h in range(ICH // 512):
                            nc.tensor.matmul(
                                sim_ps[:, h * 512:(h + 1) * 512],
                                kn_sb[:, j0:j0 + 128],
                                qn_sb[:, i0 + h * 512:i0 + (h + 1) * 512],
                                start=True, stop=True)
                        nc.scalar.activation(expT[:, jt, :], sim_ps[:],
                                             AF.Exp, scale=SCALE)
                    for h in range(ICH // 512):
                        pc = ic * (ICH // 512) + h
                        cc_in = dram.tile([D, 512], bf16,
                                          name=f"cc_in{b}_{pc}")
                        cc_out = dram.tile([DIM, 512], bf16,
                                           addr_space="Shared",
                                           name=f"cc_out{b}_{pc}")
                        cc_outs[(b, pc)] = cc_out
                        av_ps = ps.tile([D + 1, 512], f32, tag="small", bufs=4)
                        for jt in range(JT):
                            nc.tensor.matmul(
                                av_ps[:],
                                vo_sb[:, b * JT + jt, :],
                                expT[:, jt, h * 512:(h + 1) * 512],
                                start=(jt == 0), stop=(jt == JT - 1))
                        # normalize by sumexp (row D) and emit bf16
                        se_sb = sb.tile([1, 512], f32, tag="se")
                        nc.vector.tensor_copy(se_sb[:], av_ps[D:D + 1, :])
                        rse_sb = sb.tile([1, 512], f32, tag="rse")
                        nc.vector.reciprocal_approx_fast(rse_sb[:], se_sb[:])
                        r2_sb = sb.tile([D, 512], f32, tag="r2")
                        nc.gpsimd.partition_broadcast(r2_sb[:], rse_sb[0:1, :])
                        oc_sb = sb.tile([D, 512], bf16, tag="oc")
                        nc.vector.tensor_mul(oc_sb[:], av_ps[0:D, :],
                                             r2_sb[:])
                        nc.sync.dma_start(cc_in[:], oc_sb[:])
                        if collective:
                            nc.gpsimd.collective_compute(
                                "AllGather", mybir.AluOpType.bypass,
                                replica_groups=[list(range(num_devices))],
                                ins=[cc_in[:]], outs=[cc_out[:]])
                        else:
                            nc.sync.dma_start(cc_out[0:D, :], cc_in[:])

                for pc in range(NPCH):
                    cc_out = cc_outs[(b, pc)]
                    ag_sb = sb.tile([128, FT, PCH], bf16, tag="ag")
                    for t in range(FT):
                        nc.sync.dma_start(
                            ag_sb[:, t, :],
                            cc_out[t * 128:(t + 1) * 128, :])
                    fp_ps = ps.tile([D, PCH], f32, tag="small", bufs=4)
                    for t in range(FT):
                        nc.tensor.matmul(fp_ps[:], w2_sb[:, t, :],
                                         ag_sb[:, t, :],
                                         start=(t == 0), stop=(t == FT - 1))
                    fo_sb = sb.tile([D, PCH], f32, tag="fo")
                    nc.vector.tensor_copy(fo_sb[:], fp_ps[:])
                    nc.sync.dma_start(
                        outT[:, b * SEQ + pc * PCH:b * SEQ + (pc + 1) * PCH],
                        fo_sb[:])
    nc.compile()
    _BUILD_CACHE[key] = nc
    return nc


def make_in_maps(x, Wq, Wkv, Wout):
    xT = np.ascontiguousarray(x.reshape(NTOK, DIM).T).astype(np.float32)
    ones2 = np.zeros((128, 2), np.float32)
    ones2[0:D, 0] = 1.0
    ones2[D:128, 1] = 1.0
    e2 = np.ascontiguousarray(ones2.T)
    ones1 = np.ones((1, D), np.float32)
    in_maps = []
    for c in range(N_CORES):
        rows = slice(c * D, (c + 1) * D)
        wqk = np.ascontiguousarray(
            np.concatenate([Wq[rows, :].T, Wkv[rows, :].T], axis=1))
        wv = np.ascontiguousarray(Wkv[DIM + c * D:DIM + (c + 1) * D, :].T)
        w2 = np.ascontiguousarray(Wout[rows, :].T).astype(ml_dtypes.bfloat16)
        in_maps.append({
            "xT": xT, "wqk": wqk.astype(np.float32),
            "wv": wv.astype(np.float32), "w2": w2,
            "ones2": ones2, "e2": e2, "ones1": ones1,
        })
    return in_maps


def kernel(x, Wq, Wkv, Wout, _trace=False):
    nc = build()
    in_maps = make_in_maps(np.asarray(x), np.asarray(Wq), np.asarray(Wkv),
                           np.asarray(Wout))
    res = bass_utils.run_bass_kernel_spmd(
        nc, in_maps, core_ids=list(range(N_CORES)), trace=_trace)
    out = np.empty((NTOK, DIM), np.float32)
    for c in range(N_CORES):
        out[:, c * D:(c + 1) * D] = res.results[c]["outT"].T
    full = out.reshape(B, SEQ, DIM)
    if _trace:
        return full, res
    return full


# revision 13
# speedup vs baseline: 1.2692x; 1.2372x over previous
"""CosineAttention on 8 TRN2 NeuronCores.

Sharding (head + tensor parallel, per the hint):
  core c owns head h=c for both batches:
    - computes qT,kT = [Wq_h|Wk_h]^T-stationary matmuls over full xT
    - RMS-normalizes q,k in the [d, i] layout: PE ones-matmul partition
      reduction -> ACT Sqrt -> DVE reciprocal_approx_fast (SBUF-only; the
      custom DVE op mis-reads PSUM) -> PE outer-product broadcast; the
      normalize tail trails one chunk so the PE queue never stalls on it
    - simT[j,i] = kn^T qn (K=64, float32r), exp folded scale=1/8 on ACT;
      attn@v accumulation interleaved two j-tiles behind the sim stream
      so the PE keeps streaming while ACT computes exps
    - the softmax denominator rides the attn@v matmul as a ones column
      (row 64); normalize via DVE reciprocal_approx_fast + GpSimd
      partition_broadcast + DVE multiply
    - per-(batch, 512-slab) AllGather of o_cT [64, 512] (bf16)
      -> [512, 512] feature-major; both batches' attention is emitted
      before any out-proj so gathers overlap the other batch's compute
    - column-parallel out-proj: outT_c [64, n] = W2_c^T-stationary matmul
  host concatenates the 8 feature slices.

Matmul dtype: float32r (TF32-class single-pass PE mode) for the fp32 path;
bf16 for the post-softmax path (attn weights / v / out-proj operands).
"""

import numpy as np
import ml_dtypes

import concourse.bass as bass
import concourse.tile as tile
from concourse import bacc
import concourse.mybir as mybir
from concourse import bass_utils

f32 = mybir.dt.float32
f32r = mybir.dt.float32r
bf16 = mybir.dt.bfloat16
AF = mybir.ActivationFunctionType

N_CORES = 8
HEADS = 8
D = 64            # head dim
B = 2             # batch
SEQ = 2048        # tokens per batch
DIM = 512         # model dim = HEADS * D
NTOK = B * SEQ    # 4096
SCALE = D ** -0.5  # 0.125

FT = DIM // 128   # 4 f-tiles of 128
CH1 = 512         # stage-1 token chunk
NCH1 = NTOK // CH1            # 8
ICH = 1024        # phase-2 i-slab (exp batching)
NICH = SEQ // ICH             # 2 per batch
JT = SEQ // 128   # 16 j-tiles per batch
PCH = 512         # phase-3 chunk = collective slab
NPCH = SEQ // PCH             # 4 per batch
LAG = 2           # attn@v trails the sim/exp stream by this many j-tiles

_BUILD_CACHE = {}


def build(collective=True, num_devices=N_CORES):
    key = (collective, num_devices)
    if key in _BUILD_CACHE:
        return _BUILD_CACHE[key]
    nc = bacc.Bacc("TRN2", target_bir_lowering=False, debug=False,
                   num_devices=num_devices)
    xT = nc.dram_tensor("xT", [DIM, NTOK], bf16, kind="ExternalInput").ap()
    wqk = nc.dram_tensor("wqk", [DIM, 128], bf16, kind="ExternalInput").ap()
    wv = nc.dram_tensor("wv", [DIM, D], bf16, kind="ExternalInput").ap()
    w2 = nc.dram_tensor("w2", [DIM, D], bf16, kind="ExternalInput").ap()
    ones2 = nc.dram_tensor("ones2", [128, 2], f32, kind="ExternalInput").ap()
    e2 = nc.dram_tensor("e2", [2, 128], f32, kind="ExternalInput").ap()
    ones1 = nc.dram_tensor("ones1", [1, D], f32, kind="ExternalInput").ap()
    outT = nc.dram_tensor("outT", [D, NTOK], f32, kind="ExternalOutput").ap()

    with tile.TileContext(nc) as tc:
        with (
            tc.tile_pool(name="persist", bufs=1) as pp,
            tc.tile_pool(name="sb", bufs=2) as sb,
            tc.tile_pool(name="ps", bufs=1, space="PSUM") as ps,
            tc.tile_pool(name="dram", bufs=1, space="DRAM") as dram,
            nc.allow_low_precision(reason="f32r matmuls; bf16 attn/out path"),
        ):
            # ---- persistent weights / constants (w2 loads later) ----
            wqk_sb = pp.tile([128, FT, 128], bf16)
            wv_sb = pp.tile([128, FT, D], bf16)
            w2_sb = pp.tile([128, FT, D], bf16)
            for t in range(FT):
                nc.sync.dma_start(wqk_sb[:, t, :],
                                  wqk[t * 128:(t + 1) * 128, :])
                nc.sync.dma_start(wv_sb[:, t, :],
                                  wv[t * 128:(t + 1) * 128, :])
            o2_sb = pp.tile([128, 2], f32r)
            nc.sync.dma_start(o2_sb[:], ones2[:].bitcast(f32r))
            e2_sb = pp.tile([2, 128], f32r)
            nc.sync.dma_start(e2_sb[:], e2[:].bitcast(f32r))

            # ---- persistent activations ----
            qn_sb = pp.tile([D, NTOK], bf16)     # normalized qT
            kn_sb = pp.tile([D, NTOK], bf16)     # normalized kT
            qk_all = pp.tile([128, NTOK], f32r)  # raw [q;k]T
            vo_sb = pp.tile([128, NTOK // 128, D + 1], bf16)  # v | ones

            # ---- stage 1: qkv projections + rms normalization ----
            sq_tiles = {}

            def norm_tail(ci):
                cols = slice(ci * CH1, (ci + 1) * CH1)
                # st[2, CH1] = column sums of q-sq and k-sq
                st_ps = ps.tile([2, CH1], f32, tag="small", bufs=2)
                nc.tensor.matmul(st_ps[:], o2_sb[:], sq_tiles.pop(ci)[:],
                                 start=True, stop=True)
                # ssq = sqrt(st/64); rst = 1/ssq  (eps dropped: ~1e-4 rel)
                ssq_sb = sb.tile([2, CH1], f32, tag="ssq")
                nc.scalar.activation(ssq_sb[:], st_ps[:], AF.Sqrt,
                                     scale=1.0 / D)
                rst_sb = sb.tile([2, CH1], f32, tag="rst")
                nc.vector.reciprocal_approx_fast(rst_sb[:], ssq_sb[:])
                rstr_sb = sb.tile([2, CH1], f32r, tag="rstr")
                nc.vector.tensor_copy(rstr_sb[:], rst_sb[:])
                # broadcast scales across head-dim partitions (PE outer
                # product); muls consume the PSUM broadcast directly
                r_ps = ps.tile([128, CH1], f32, tag="small", bufs=2)
                nc.tensor.matmul(r_ps[:], e2_sb[:], rstr_sb[:],
                                 start=True, stop=True)
                nc.vector.tensor_mul(qn_sb[:, cols],
                                     qk_all[0:D, cols].bitcast(f32),
                                     r_ps[0:D, :])
                nc.vector.tensor_mul(kn_sb[:, cols],
                                     qk_all[D:128, cols].bitcast(f32),
                                     r_ps[D:128, :])

            for ci in range(NCH1):
                cols = slice(ci * CH1, (ci + 1) * CH1)
                xt_sb = sb.tile([128, FT, CH1], bf16, tag="xt")
                for t in range(FT):
                    nc.sync.dma_start(
                        xt_sb[:, t, :],
                        xT[t * 128:(t + 1) * 128, cols])

                # qkT chunk: [q;k] x-stream, W stationary
                qk_ps = ps.tile([128, CH1], f32, tag="small", bufs=2)
                for t in range(FT):
                    nc.tensor.matmul(qk_ps[:], wqk_sb[:, t, :], xt_sb[:, t, :],
                                     start=(t == 0), stop=(t == FT - 1))
                # v chunk: [j, d], xT stationary
                for js in range(CH1 // 128):
                    v_ps = ps.tile([128, D], f32, tag="small", bufs=2)
                    for t in range(FT):
                        nc.tensor.matmul(
                            v_ps[:],
                            xt_sb[:, t, js * 128:(js + 1) * 128],
                            wv_sb[:, t, :],
                            start=(t == 0), stop=(t == FT - 1))
                    jt = ci * (CH1 // 128) + js
                    nc.vector.tensor_copy(vo_sb[:, jt, 0:D], v_ps[:])
                    nc.gpsimd.memset(vo_sb[:, jt, D:D + 1], 1.0)

                # raw qk to sbuf (DVE), square for the partition reduce
                nc.vector.tensor_copy(qk_all[:, cols], qk_ps[:])
                sq_sb = sb.tile([128, CH1], f32r, tag="sq")
                nc.vector.tensor_mul(sq_sb[:], qk_all[:, cols],
                                     qk_all[:, cols])
                sq_tiles[ci] = sq_sb
                # the PE part of chunk ci-1's normalize goes behind chunk
                # ci's matmuls so the PE never waits on the DVE chain
                if ci >= 1:
                    norm_tail(ci - 1)
            norm_tail(NCH1 - 1)

            # ---- attention for both batches, then both out-projs ----
            cc_outs = {}
            for b in range(B):
                for ic in range(NICH):
                    i0 = b * SEQ + ic * ICH
                    expT = sb.tile([128, JT, ICH], bf16, tag="expT")
                    av_ps = [ps.tile([D + 1, 512], f32, tag="av", bufs=2,
                                     name=f"av{b}_{ic}_{h2}")
                             for h2 in range(ICH // 512)]

                    def av_step(jt):
                        for h2 in range(ICH // 512):
                            nc.tensor.matmul(
                                av_ps[h2][:],
                                vo_sb[:, b * JT + jt, :],
                                expT[:, jt, h2 * 512:(h2 + 1) * 512],
                                start=(jt == 0), stop=(jt == JT - 1))

                    for jt in range(JT):
                        j0 = b * SEQ + jt * 128
                        sim_ps = ps.tile([128, ICH], f32, tag="big", bufs=2)
                        for h in range(ICH // 512):
                            nc.tensor.matmul(
                                sim_ps[:, h * 512:(h + 1) * 512],
                                kn_sb[:, j0:j0 + 128],
                                qn_sb[:, i0 + h * 512:i0 + (h + 1) * 512],
                                start=True, stop=True)
                        nc.scalar.activation(expT[:, jt, :], sim_ps[:],
                                             AF.Exp, scale=SCALE)
                        if jt >= LAG:
                            av_step(jt - LAG)
                    for jt in range(JT - LAG, JT):
                        av_step(jt)

                    for h in range(ICH // 512):
                        pc = ic * (ICH // 512) + h
                        cc_in = dram.tile([D, 512], bf16,
                                          name=f"cc_in{b}_{pc}")
                        cc_out = dram.tile([DIM, 512], bf16,
                                           addr_space="Shared",
                                           name=f"cc_out{b}_{pc}")
                        cc_outs[(b, pc)] = cc_out
                        # normalize by sumexp (row D) and emit bf16
                        se_sb = sb.tile([1, 512], f32, tag="se")
                        nc.vector.tensor_copy(se_sb[:], av_ps[h][D:D + 1, :])
                        rse_sb = sb.tile([1, 512], f32, tag="rse")
                        nc.vector.reciprocal_approx_fast(rse_sb[:], se_sb[:])
                        r2_sb = sb.tile([D, 512], f32, tag="r2")
                        nc.gpsimd.partition_broadcast(r2_sb[:], rse_sb[0:1, :])
                        oc_sb = sb.tile([D, 512], bf16, tag="oc")
                        nc.vector.tensor_mul(oc_sb[:], av_ps[h][0:D, :],
                                             r2_sb[:])
                        nc.gpsimd.dma_start(cc_in[:], oc_sb[:])
                        if collective:
                            nc.gpsimd.collective_compute(
                                "AllGather", mybir.AluOpType.bypass,
                                replica_groups=[list(range(num_devices))],
                                ins=[cc_in[:]], outs=[cc_out[:]])
                        else:
                            nc.gpsimd.dma_start(cc_out[0:D, :], cc_in[:])

            # out-proj weights only needed now
            for t in range(FT):
                nc.gpsimd.dma_start(w2_sb[:, t, :],
                                    w2[t * 128:(t + 1) * 128, :])
            for b in range(B):
                for pc in range(NPCH):
                    cc_out = cc_outs[(b, pc)]
                    ag_sb = sb.tile([128, FT, PCH], bf16, tag="ag")
                    for t in range(FT):
                        nc.gpsimd.dma_start(
                            ag_sb[:, t, :],
                            cc_out[t * 128:(t + 1) * 128, :])
                    fp_ps = ps.tile([D, PCH], f32, tag="small", bufs=2)
                    for t in range(FT):
                        nc.tensor.matmul(fp_ps[:], w2_sb[:, t, :],
                                         ag_sb[:, t, :],
                                         start=(t == 0), stop=(t == FT - 1))
                    fo_sb = sb.tile([D, PCH], f32, tag="fo")
                    nc.vector.tensor_copy(fo_sb[:], fp_ps[:])
                    nc.gpsimd.dma_start(
                        outT[:, b * SEQ + pc * PCH:b * SEQ + (pc + 1) * PCH],
                        fo_sb[:])
    nc.compile()
    _BUILD_CACHE[key] = nc
    return nc


def make_in_maps(x, Wq, Wkv, Wout):
    xT = np.ascontiguousarray(x.reshape(NTOK, DIM).T).astype(
        ml_dtypes.bfloat16)
    ones2 = np.zeros((128, 2), np.float32)
    ones2[0:D, 0] = 1.0
    ones2[D:128, 1] = 1.0
    e2 = np.ascontiguousarray(ones2.T)
    ones1 = np.ones((1, D), np.float32)
    in_maps = []
    for c in range(N_CORES):
        rows = slice(c * D, (c + 1) * D)
        wqk = np.ascontiguousarray(
            np.concatenate([Wq[rows, :].T, Wkv[rows, :].T], axis=1))
        wv = np.ascontiguousarray(Wkv[DIM + c * D:DIM + (c + 1) * D, :].T)
        w2 = np.ascontiguousarray(Wout[rows, :].T).astype(ml_dtypes.bfloat16)
        in_maps.append({
            "xT": xT, "wqk": wqk.astype(ml_dtypes.bfloat16),
            "wv": wv.astype(ml_dtypes.bfloat16), "w2": w2,
            "ones2": ones2, "e2": e2, "ones1": ones1,
        })
    return in_maps


def kernel(x, Wq, Wkv, Wout, _trace=False):
    nc = build()
    in_maps = make_in_maps(np.asarray(x), np.asarray(Wq), np.asarray(Wkv),
                           np.asarray(Wout))
    res = bass_utils.run_bass_kernel_spmd(
        nc, in_maps, core_ids=list(range(N_CORES)), trace=_trace)
    out = np.empty((NTOK, DIM), np.float32)
    for c in range(N_CORES):
        out[:, c * D:(c + 1) * D] = res.results[c]["outT"].T
    full = out.reshape(B, SEQ, DIM)
    if _trace:
        return full, res
    return full
